# revision 1
# baseline (speedup 1.0000x reference)
"""Trainium2 Bass kernel for nn_L2PppMaskAttn (topk_masking).

Math reformulation of the reference:
  - a_k = sum(l2norm(K[idx]) * l2norm(A[idx])) depends only on (layer, prompt):
    precompute s[l,p] = <K_hat[l,p], A_hat[l,p]> once per layer.
  - top-5 ranking over prompts is invariant to q normalization (positive
    per-row scale), so scores u[b,p] = <x[b,l], K_hat[l,p]> suffice.
  - out[l,b] = sum_{p in top5} s[l,p] * P[l,p] = (mask_row .* s) @ P_flat[l],
    a dense [B,100] @ [100, 6144] matmul per layer (topk -> masking).

Sharding: data-parallel over batch, 8 cores x 128 rows; K/A/P replicated.
"""

import sys

sys.path.insert(0, "/opt/trn_rl_repo")

import numpy as np

B, L, P_N, LP, D = 1024, 12, 100, 8, 768
N_CORES = 8
BS = B // N_CORES  # 128 batch rows per core
NF = LP * D  # 6144 flattened output features per layer
TOP_K = 5
NEG_BIG = -1.0e30

_CACHE = {}


def _build_nc():
    if "nc" in _CACHE:
        return _CACHE["nc"]

    from contextlib import ExitStack

    import concourse.bass as bass
    import concourse.bacc as bacc
    import concourse.mybir as mybir
    from concourse import masks
    from concourse.tile import TileContext

    f32 = mybir.dt.float32
    f32r = mybir.dt.float32r
    AX = mybir.AxisListType
    OP = mybir.AluOpType
    AF = mybir.ActivationFunctionType

    nc = bacc.Bacc(
        "TRN2",
        target_bir_lowering=False,
        debug=False,
        num_devices=N_CORES,
    )

    x_d = nc.declare_dram_parameter("x", [BS, L * D], f32, isOutput=False)
    k_d = nc.declare_dram_parameter("k", [L, P_N, D], f32, isOutput=False)
    a_d = nc.declare_dram_parameter("a", [L, P_N, D], f32, isOutput=False)
    p_d = nc.declare_dram_parameter("p", [L, P_N, NF], f32, isOutput=False)
    o_d = nc.declare_dram_parameter("o", [L, BS, NF], f32, isOutput=True)

    with TileContext(nc) as tc, ExitStack() as ctx:
        pool = lambda name, bufs, **kw: ctx.enter_context(
            tc.tile_pool(name=name, bufs=bufs, **kw)
        )
        const = pool("const", 1)
        xp = pool("xp", 2)
        kap = pool("kap", 2)
        scrp = pool("scrp", 2)
        nrm = pool("nrm", 2)
        nktp = pool("nktp", 2)
        xtp = pool("xtp", 2)
        ppool = pool("pp", 2)
        obuf = pool("ob", 2)
        small = pool("small", 3)
        rowp = pool("rowp", 2)
        wtp = pool("wtp", 2)
        ps_t = pool("ps_t", 2, space="PSUM")
        ps_c = pool("ps_c", 2, space="PSUM")
        ps_o = pool("ps_o", 4, space="PSUM")

        ident = const.tile([128, 128], f32)
        masks.make_identity(nc, ident[:])

        x_dv = x_d[:].rearrange("b (l d) -> b l d", l=L)

        for l in range(L):
            # ---- load pools for this layer ----
            x_sb = xp.tile([BS, D], f32)
            nc.sync.dma_start(x_sb[:], x_dv[:, l])
            ka = kap.tile([P_N, D], f32, tag="ka")
            nc.sync.dma_start(ka[:], k_d[l])
            aa = kap.tile([P_N, D], f32, tag="aa")
            nc.sync.dma_start(aa[:], a_d[l])
            # f32r (TF32-class) pool operand: rounded during the DMA cast,
            # runs the output matmul at 1 cycle/row instead of fp32's 4.
            p_sb = ppool.tile([P_N, NF], f32r)
            nc.gpsimd.dma_start(p_sb[:], p_d[l])

            # ---- l2 norms of K and A rows: rsqrt(sum(sq)) w/ Newton polish ----
            rs = []
            for src in (ka, aa):
                scr = scrp.tile([P_N, D], f32, tag="scr")
                ss = small.tile([P_N, 1], f32, tag="ss")
                nc.scalar.activation(scr[:], src[:], AF.Square, accum_out=ss[:])
                sq = small.tile([P_N, 1], f32, tag="sq")
                nc.scalar.activation(sq[:], ss[:], AF.Sqrt)
                y0 = small.tile([P_N, 1], f32, tag="y0")
                nc.vector.reciprocal(y0[:], sq[:])
                # one Newton step: y = y0 * (1.5 - 0.5 * ss * y0^2)
                t1 = small.tile([P_N, 1], f32, tag="t1")
                nc.vector.tensor_tensor(t1[:], y0[:], y0[:], op=OP.mult)
                nc.vector.tensor_tensor(t1[:], t1[:], ss[:], op=OP.mult)
                nc.vector.tensor_scalar(t1[:], t1[:], -0.5, 1.5, OP.mult, OP.add)
                y1 = small.tile([P_N, 1], f32, tag="y1")
                nc.vector.tensor_tensor(y1[:], t1[:], y0[:], op=OP.mult)
                rs.append(y1)

            nk = nrm.tile([P_N, D], f32, tag="nk")
            nc.vector.tensor_scalar_mul(nk[:], ka[:], rs[0][:])
            na = nrm.tile([P_N, D], f32, tag="na")
            nc.vector.tensor_scalar_mul(na[:], aa[:], rs[1][:])

            # s[p] = <nk_p, na_p>
            scr2 = scrp.tile([P_N, D], f32, tag="scr")
            s_t = small.tile([P_N, 1], f32, tag="s_t")
            nc.vector.tensor_tensor(scr2[:], nk[:], na[:], op=OP.mult)
            nc.vector.reduce_sum(s_t[:], scr2[:], axis=AX.X)

            # ---- transpose nk -> [768(=6x128), 100] and x[:, l] -> [768, 128] ----
            nkt = nktp.tile([128, 6 * P_N], f32)
            for j in range(6):
                pt = ps_t.tile([128, P_N], f32, tag="tp")
                nc.tensor.transpose(
                    pt[:], nk[:, j * 128 : (j + 1) * 128], ident[:P_N, :P_N]
                )
                nc.scalar.copy(nkt[:, j * P_N : (j + 1) * P_N], pt[:])
            xt = xtp.tile([128, D], f32)
            for j in range(6):
                pt = ps_t.tile([128, 128], f32, tag="tp")
                nc.tensor.transpose(
                    pt[:], x_sb[:, j * 128 : (j + 1) * 128], ident[:]
                )
                nc.scalar.copy(xt[:, j * 128 : (j + 1) * 128], pt[:])

            # ---- scores u = x_l @ nk.T : psum [128b, 100p] ----
            pc = ps_c.tile([BS, P_N], f32)
            for j in range(6):
                nc.tensor.matmul(
                    pc[:],
                    xt[:, j * 128 : (j + 1) * 128],
                    nkt[:, j * P_N : (j + 1) * P_N],
                    start=(j == 0),
                    stop=(j == 5),
                )
            cos = rowp.tile([BS, P_N], f32, tag="cos")
            nc.scalar.copy(cos[:], pc[:])
            work = rowp.tile([BS, P_N], f32, tag="work")
            nc.vector.tensor_copy(work[:], cos[:])

            # ---- iterative top-5: find 5th max per row ----
            mm = small.tile([BS, TOP_K], f32, tag="mm")
            pen = rowp.tile([BS, P_N], f32, tag="pen")
            for it in range(TOP_K):
                nc.vector.reduce_max(mm[:, it : it + 1], work[:], axis=AX.X)
                if it < TOP_K - 1:
                    nc.vector.tensor_scalar(
                        pen[:], work[:], mm[:, it : it + 1], NEG_BIG, OP.is_ge, OP.mult
                    )
                    nc.vector.tensor_tensor(work[:], work[:], pen[:], op=OP.add)

            # mask = (u >= t5) in {0,1}
            mask = rowp.tile([BS, P_N], f32, tag="mask")
            nc.vector.tensor_scalar(
                mask[:], cos[:], mm[:, TOP_K - 1 : TOP_K], None, OP.is_ge
            )

            # W^T = mask^T * s  -> [100, 128]
            mt = ps_t.tile([P_N, 128], f32, tag="tp")
            nc.tensor.transpose(mt[:], mask[:], ident[:])
            wt = wtp.tile([P_N, BS], f32r)
            nc.vector.tensor_scalar_mul(wt[:], mt[:], s_t[:])

            # ---- out[l] = W @ P_flat : 12 x [128, 512] matmuls ----
            ob = obuf.tile([BS, NF], f32)
            for n in range(12):
                po = ps_o.tile([BS, 512], f32)
                nc.tensor.matmul(
                    po[:], wt[:], p_sb[:, n * 512 : (n + 1) * 512], start=True, stop=True
                )
                if n % 2 == 0:
                    nc.scalar.copy(ob[:, n * 512 : (n + 1) * 512], po[:])
                else:
                    nc.vector.tensor_copy(ob[:, n * 512 : (n + 1) * 512], po[:])
            nc.sync.dma_start(o_d[l], ob[:])

    nc.compile()
    _CACHE["nc"] = nc
    return nc


def _run(x_query, K_all, A_all, P_all, trace=False, tmpdir=None):
    from concourse.bass_utils import run_bass_kernel_spmd

    x = np.ascontiguousarray(np.asarray(x_query, dtype=np.float32)).reshape(B, L * D)
    k = np.ascontiguousarray(np.asarray(K_all, dtype=np.float32))
    a = np.ascontiguousarray(np.asarray(A_all, dtype=np.float32))
    p = np.ascontiguousarray(np.asarray(P_all, dtype=np.float32)).reshape(L, P_N, NF)

    nc = _build_nc()
    in_maps = [
        {"x": x[c * BS : (c + 1) * BS], "k": k, "a": a, "p": p} for c in range(N_CORES)
    ]
    br = run_bass_kernel_spmd(
        nc, in_maps, list(range(N_CORES)), trace=trace, tmpdir=tmpdir
    )
    out = np.stack([r["o"] for r in br.results], axis=0)  # [8, L, BS, NF]
    out = out.transpose(1, 0, 2, 3).reshape(L, B, LP, D)
    return out, br


def kernel(x_query, K_all, A_all, P_all):
    out, _ = _run(x_query, K_all, A_all, P_all)
    return out



# revision 2
# speedup vs baseline: 1.0548x; 1.0548x over previous
"""Trainium2 Bass kernel for nn_L2PppMaskAttn (topk_masking).

Math reformulation of the reference:
  - a_k = sum(l2norm(K[idx]) * l2norm(A[idx])) depends only on (layer, prompt):
    precompute s[l,p] = <K,A> / (||K|| ||A||) once per layer on-device.
  - top-5 ranking over prompts is invariant to q normalization (positive
    per-row scale), so scores u[b,p] = <x[b,l], K[l,p]> / ||K[l,p]|| suffice.
  - out[l,b] = sum_{p in top5} s[l,p] * P[l,p] = (mask_row .* s) @ P_flat[l],
    a dense [B,100] @ [100, 6144] matmul per layer (topk -> masking).

v2 layout/pipeline rework (356us -> target ~240us, memory roofline ~222us):
  - Host passes x, K, A pre-transposed (d on partitions) so no PE
    transposes are needed on-chip; norms/<K,A> reduce over the partition
    dim via ones-vector matmuls after an on-chip square/multiply+fold.
  - DMA ring split to avoid HWDGE FIFO head-of-line blocking:
    stores on nc.sync (SP ring), P-pool loads on nc.scalar (ACT ring,
    dram tensor declared f32r so no SWDGE cast needed), small x/K/A
    loads on nc.gpsimd (SWDGE).
  - Output written as two [128,3072] half-stores per layer for finer
    store/compute overlap; deeper buffering (bufs=3 on streamed pools).

Sharding: data-parallel over batch, 8 cores x 128 rows; K/A/P replicated.
"""

import sys

sys.path.insert(0, "/opt/trn_rl_repo")

import numpy as np

B, L, P_N, LP, D = 1024, 12, 100, 8, 768
N_CORES = 8
BS = B // N_CORES  # 128 batch rows per core
NF = LP * D  # 6144 flattened output features per layer
NC6 = D // 128  # 6 contraction chunks of 128
TOP_K = 5
NEG_BIG = -1.0e30

_CACHE = {}


def _build_nc():
    if "nc" in _CACHE:
        return _CACHE["nc"]

    from contextlib import ExitStack

    import concourse.bass as bass
    import concourse.bacc as bacc
    import concourse.mybir as mybir
    from concourse import masks
    from concourse.tile import TileContext

    f32 = mybir.dt.float32
    f32r = mybir.dt.float32r
    AX = mybir.AxisListType
    OP = mybir.AluOpType
    AF = mybir.ActivationFunctionType

    nc = bacc.Bacc(
        "TRN2",
        target_bir_lowering=False,
        debug=False,
        num_devices=N_CORES,
    )

    # Host-side layouts (see _run):
    #  xt[l, p, c*128 + b]   = x_core[b, l, c*128 + p]
    #  kat[l, p, c*100 + j]  = K[l, j, c*128 + p]          (cols 0..599)
    #  kat[l, p, 600 + c*100 + j] = A[l, j, c*128 + p]     (cols 600..1199)
    #  p[l, j, :]            = P[l, j].reshape(NF)
    xt_d = nc.declare_dram_parameter("xt", [L, BS, D], f32, isOutput=False)
    kat_d = nc.declare_dram_parameter("kat", [L, 128, 2 * NC6 * P_N], f32, isOutput=False)
    p_d = nc.declare_dram_parameter("p", [L, P_N, NF], f32r, isOutput=False)
    o_d = nc.declare_dram_parameter("o", [L, BS, NF], f32, isOutput=True)

    with TileContext(nc) as tc, ExitStack() as ctx:
        pool = lambda name, bufs, **kw: ctx.enter_context(
            tc.tile_pool(name=name, bufs=bufs, **kw)
        )
        const = pool("const", 1)
        katp = pool("katp", 3)
        xtp = pool("xtp", 3)
        ppool = pool("pp", 3)
        sqp = pool("sqp", 2)
        foldp = pool("foldp", 2)
        smp = pool("smp", 3)
        obp = pool("obp", 4)
        ps_sp = pool("ps_sp", 2, space="PSUM")   # red / rkr / mt (shared tag)
        ps_bc = pool("ps_bc", 1, space="PSUM")
        ps_sc = pool("ps_sc", 1, space="PSUM")
        ps_o = pool("ps_o", 4, space="PSUM")

        ident = const.tile([128, 128], f32)
        masks.make_identity(nc, ident[:])
        ones_col = const.tile([128, 1], f32)
        nc.vector.memset(ones_col[:], 1.0)
        ones_row = const.tile([1, 128], f32)
        nc.vector.memset(ones_row[:], 1.0)

        for l in range(L):
            # ---- loads: small tensors on SWDGE, P pool on ACT HWDGE ----
            kat = katp.tile([128, 2 * NC6 * P_N], f32)
            nc.gpsimd.dma_start(kat[:], kat_d[l])
            xt = xtp.tile([BS, D], f32)
            nc.gpsimd.dma_start(xt[:], xt_d[l])
            p_sb = ppool.tile([P_N, NF], f32r)
            nc.scalar.dma_start(p_sb[:], p_d[l])

            # ---- partial products: K^2 | A^2 | K*A  (d on partitions) ----
            sq = sqp.tile([128, 1800], f32)
            nc.scalar.activation(sq[:, 0:600], kat[:, 0:600], AF.Square)
            nc.scalar.activation(sq[:, 600:1200], kat[:, 600:1200], AF.Square)
            nc.vector.tensor_tensor(
                sq[:, 1200:1800], kat[:, 0:600], kat[:, 600:1200], op=OP.mult
            )

            # ---- fold the 6 d-chunks of each quantity: [128,1800]->[128,300] ----
            tq = foldp.tile([128, 900], f32, tag="tq")
            f300 = foldp.tile([128, 300], f32, tag="f300")
            for q in range(3):
                b6 = q * 600
                nc.vector.tensor_tensor(
                    tq[:, q * 300 : q * 300 + 300],
                    sq[:, b6 : b6 + 300],
                    sq[:, b6 + 300 : b6 + 600],
                    op=OP.add,
                )
                nc.vector.tensor_tensor(
                    f300[:, q * 100 : (q + 1) * 100],
                    tq[:, q * 300 : q * 300 + 100],
                    tq[:, q * 300 + 100 : q * 300 + 200],
                    op=OP.add,
                )
                nc.vector.tensor_tensor(
                    f300[:, q * 100 : (q + 1) * 100],
                    f300[:, q * 100 : (q + 1) * 100],
                    tq[:, q * 300 + 200 : q * 300 + 300],
                    op=OP.add,
                )

            # ---- partition-dim reduction via ones matmul: [100,3] cols ----
            red = ps_sp.tile([P_N, 4], f32, tag="sp1")
            for q in range(3):
                nc.tensor.matmul(
                    red[:, q : q + 1],
                    f300[:, q * 100 : (q + 1) * 100],
                    ones_col[:],
                    start=True,
                    stop=True,
                )
            sqs = smp.tile([P_N, 3], f32, tag="sqs")
            nc.vector.tensor_copy(sqs[:], red[:, 0:3])

            # ---- rsqrt of ||K||^2, ||A||^2 with one Newton step ----
            srt = smp.tile([P_N, 2], f32, tag="srt")
            nc.scalar.activation(srt[:], sqs[:, 0:2], AF.Sqrt)
            y0 = smp.tile([P_N, 2], f32, tag="y0")
            nc.vector.reciprocal(y0[:], srt[:])
            t1 = smp.tile([P_N, 2], f32, tag="t1")
            nc.vector.tensor_tensor(t1[:], y0[:], y0[:], op=OP.mult)
            nc.vector.tensor_tensor(t1[:], t1[:], sqs[:, 0:2], op=OP.mult)
            nc.vector.tensor_scalar(t1[:], t1[:], -0.5, 1.5, OP.mult, OP.add)
            rs2 = smp.tile([P_N, 2], f32, tag="rs2")
            nc.vector.tensor_tensor(rs2[:], t1[:], y0[:], op=OP.mult)

            # s[p] = <K,A> * rsK * rsA   (column form, for the wt scale)
            s_col = smp.tile([P_N, 1], f32, tag="scol")
            nc.vector.tensor_tensor(s_col[:], rs2[:, 0:1], rs2[:, 1:2], op=OP.mult)
            nc.vector.tensor_tensor(s_col[:], s_col[:], sqs[:, 2:3], op=OP.mult)

            # rsK as a row, broadcast to [128,100] via outer product
            rkr_ps = ps_sp.tile([1, P_N], f32, tag="sp1")
            nc.tensor.transpose(rkr_ps[:], rs2[:, 0:1], ident[:P_N, :P_N])
            rkr_sb = smp.tile([1, P_N], f32, tag="rkrsb")
            nc.scalar.copy(rkr_sb[:], rkr_ps[:])
            bc_ps = ps_bc.tile([BS, P_N], f32)
            nc.tensor.matmul(bc_ps[:], ones_row[:], rkr_sb[:], start=True, stop=True)
            bc_sb = smp.tile([BS, P_N], f32, tag="bcsb")
            nc.vector.tensor_copy(bc_sb[:], bc_ps[:])

            # ---- raw scores x @ K^T accumulated over 6 d-chunks ----
            sc_ps = ps_sc.tile([BS, P_N], f32)
            for c in range(NC6):
                nc.tensor.matmul(
                    sc_ps[:],
                    xt[:, c * 128 : (c + 1) * 128],
                    kat[:, c * 100 : (c + 1) * 100],
                    start=(c == 0),
                    stop=(c == NC6 - 1),
                )
            S = smp.tile([BS, P_N], f32, tag="S")
            nc.vector.tensor_tensor(S[:], sc_ps[:], bc_sb[:], op=OP.mult)

            # ---- iterative top-5: find 5th max per row ----
            mm = smp.tile([BS, TOP_K], f32, tag="mm")
            pen = smp.tile([BS, P_N], f32, tag="pen")
            work = smp.tile([BS, P_N], f32, tag="work")
            nc.vector.reduce_max(mm[:, 0:1], S[:], axis=AX.X)
            for it in range(1, TOP_K):
                src = S if it == 1 else work
                nc.vector.tensor_scalar(
                    pen[:], src[:], mm[:, it - 1 : it], NEG_BIG, OP.is_ge, OP.mult
                )
                nc.vector.tensor_tensor(work[:], src[:], pen[:], op=OP.add)
                nc.vector.reduce_max(mm[:, it : it + 1], work[:], axis=AX.X)

            # mask = (S >= t5) in {0,1}; W^T = mask^T * s -> [100, 128]
            mask = smp.tile([BS, P_N], f32, tag="mask")
            nc.vector.tensor_scalar(
                mask[:], S[:], mm[:, TOP_K - 1 : TOP_K], None, OP.is_ge
            )
            mt = ps_sp.tile([P_N, BS], f32, tag="sp1")
            nc.tensor.transpose(mt[:], mask[:], ident[:])
            wt = smp.tile([P_N, BS], f32r, tag="wt")
            nc.vector.tensor_scalar_mul(wt[:], mt[:], s_col[:])

            # ---- out[l] = W @ P_flat : two half-layer stores ----
            for h in range(2):
                ob = obp.tile([BS, NF // 2], f32)
                for j in range(6):
                    n = h * 6 + j
                    po = ps_o.tile([BS, 512], f32)
                    nc.tensor.matmul(
                        po[:],
                        wt[:],
                        p_sb[:, n * 512 : (n + 1) * 512],
                        start=True,
                        stop=True,
                    )
                    if j % 3 == 2:
                        nc.scalar.copy(ob[:, j * 512 : (j + 1) * 512], po[:])
                    else:
                        nc.vector.tensor_copy(ob[:, j * 512 : (j + 1) * 512], po[:])
                nc.sync.dma_start(
                    o_d[l][:, h * (NF // 2) : (h + 1) * (NF // 2)], ob[:]
                )

    nc.compile()
    _CACHE["nc"] = nc
    return nc


def _pack_inputs(x_query, K_all, A_all, P_all):
    x = np.ascontiguousarray(np.asarray(x_query, dtype=np.float32))
    k = np.asarray(K_all, dtype=np.float32)
    a = np.asarray(A_all, dtype=np.float32)
    p = np.ascontiguousarray(
        np.asarray(P_all, dtype=np.float32).reshape(L, P_N, NF)
    )

    def t_pool(m):  # [L,P,D] -> [L,128,6*P]: out[l,p,c*P+j] = m[l,j,c*128+p]
        r = m.transpose(0, 2, 1).reshape(L, NC6, 128, P_N)
        return r.transpose(0, 2, 1, 3).reshape(L, 128, NC6 * P_N)

    kat = np.ascontiguousarray(
        np.concatenate([t_pool(k), t_pool(a)], axis=2)
    )

    xts = []
    for c in range(N_CORES):
        xc = x[c * BS : (c + 1) * BS]  # [128, L, D]
        # xt[l,p,c6*128+b] = xc[b,l,c6*128+p]
        r = xc.transpose(1, 2, 0).reshape(L, NC6, 128, BS)
        xts.append(
            np.ascontiguousarray(r.transpose(0, 2, 1, 3).reshape(L, BS, D))
        )
    return xts, kat, p


def _run(x_query, K_all, A_all, P_all, trace=False, tmpdir=None):
    from concourse.bass_utils import run_bass_kernel_spmd

    xts, kat, p = _pack_inputs(x_query, K_all, A_all, P_all)

    nc = _build_nc()
    in_maps = [{"xt": xts[c], "kat": kat, "p": p} for c in range(N_CORES)]
    br = run_bass_kernel_spmd(
        nc, in_maps, list(range(N_CORES)), trace=trace, tmpdir=tmpdir
    )
    out = np.stack([r["o"] for r in br.results], axis=0)  # [8, L, BS, NF]
    out = out.transpose(1, 0, 2, 3).reshape(L, B, LP, D)
    return out, br


def kernel(x_query, K_all, A_all, P_all):
    out, _ = _run(x_query, K_all, A_all, P_all)
    return out
